# revision 1
# baseline (speedup 1.0000x reference)
"""DeepseekV3 MLA prefill attention on 8 Trainium2 NeuronCores.

Strategy (no on-device collectives; host does shard/gather data movement only):

  Launch 1 (row-sharded): each core takes S/8=256 sequence rows and computes the
    low-rank "a" projections for its rows: lq = x @ dq(wq_a).T -> rmsnorm,
    lkv = x @ dq(wkv_a).T -> rmsnorm(kv part) + rope(k_pe part).
    Weights wq_a/wkv_a are replicated (reading them once per core is unavoidable
    without cross-core comms; compute is small).

  Host: gathers the 8 row-shards, transposes to feature-major layout.

  Launch 2 (head-sharded, 2 of 16 heads per core): q_b / kv_b projections for the
    core's heads (column-parallel), causal attention in transposed layout
    (scores^T = K^T-tiles x Q^T, softmax without max-subtraction -- scores are
    provably small for this distribution -- unnormalized probs, PV accumulation,
    normalization by matmul-computed column sums), then the o_proj row-parallel
    partial product for the core's 256 input columns.

  Host: sums the 8 o_proj partials (the all-reduce of the sharding hint, done at
    the gather step) and transposes back to [S, HID].

All matmuls run as fp32r (full-rate fp32 on the PE at N>=256). Dequantization of
the int8-block-scaled weights happens on device via broadcast-AP tensor ops.
"""

import math
import os

import numpy as np

import concourse.bass as bass
import concourse.bacc as bacc
import concourse.mybir as mybir
import concourse.tile as tile
from concourse.bass import ts, ds
from concourse.bass_utils import run_bass_kernel_spmd

F32 = mybir.dt.float32
F32R = mybir.dt.float32r
AF = mybir.ActivationFunctionType
ALU = mybir.AluOpType

S, HID = 2048, 2048
NH, Q_LORA, KV_LORA = 16, 1536, 512
NOPE, ROPE, VDIM = 128, 64, 128
HEAD = NOPE + ROPE            # 192
NC_N = 8                      # cores
HPC = NH // NC_N              # heads per core = 2
R = S // NC_N                 # rows per core in L1 = 256
EPS = 1e-6
MSCALE = 0.1 * 1.0 * math.log(40.0) + 1.0
SOFTMAX_SCALE = HEAD ** -0.5 * MSCALE * MSCALE

P = 128
SQB = 512                     # q-column block in attention
NSQB = S // SQB               # 4
NSKT = S // P                 # 16 sk tiles


def _bcast_ap(ap, p=P):
    """DRAM/SBUF AP broadcast across p partitions (step-0 partition dim)."""
    return bass.AP(tensor=ap.tensor, offset=ap.offset, ap=[[0, p]] + list(ap.ap))


# --------------------------------------------------------------------------
# Launch 1: row-sharded a-projections + rmsnorm + k_pe rope
# --------------------------------------------------------------------------

def build_l1(reps=1):
    nc = bacc.Bacc("TRN2", debug=False, num_devices=NC_N)
    xT = nc.dram_tensor("xT", [HID, R], F32R, kind="ExternalInput").ap()
    wqaT = nc.dram_tensor("wqaT", [HID, Q_LORA], F32R, kind="ExternalInput").ap()
    wkvaT = nc.dram_tensor("wkvaT", [HID, KV_LORA + ROPE], F32R, kind="ExternalInput").ap()
    sqa = nc.dram_tensor("sqa", [12, 16], F32, kind="ExternalInput").ap()
    skva = nc.dram_tensor("skva", [5, 16], F32, kind="ExternalInput").ap()
    qlnw = nc.dram_tensor("qlnw", [1, Q_LORA], F32, kind="ExternalInput").ap()
    kvlnw = nc.dram_tensor("kvlnw", [1, KV_LORA], F32, kind="ExternalInput").ap()
    cosr = nc.dram_tensor("cosr", [R, ROPE], F32, kind="ExternalInput").ap()
    sinr = nc.dram_tensor("sinr", [R, ROPE], F32, kind="ExternalInput").ap()
    lnq = nc.dram_tensor("lnq", [R, Q_LORA], F32, kind="ExternalOutput").ap()
    lnkv = nc.dram_tensor("lnkv", [R, KV_LORA], F32, kind="ExternalOutput").ap()
    kpe = nc.dram_tensor("kpe", [R, ROPE], F32, kind="ExternalOutput").ap()

    KT = HID // P   # 16 contraction tiles
    MT = R // P     # 2 row tiles

    with tile.TileContext(nc) as tc:
      for _rep in range(reps):
        with tc.tile_pool(name="wq", bufs=1) as wqp, \
             tc.tile_pool(name="wkv", bufs=1) as wkvp, \
             tc.tile_pool(name="xp", bufs=1) as xp, \
             tc.tile_pool(name="small", bufs=1) as smallp, \
             tc.tile_pool(name="stat", bufs=8) as statp, \
             tc.tile_pool(name="scratch", bufs=2) as scrp, \
             tc.tile_pool(name="outp", bufs=2) as outp, \
             tc.tile_pool(name="psq", bufs=2, space="PSUM") as psqp, \
             tc.tile_pool(name="pskv", bufs=1, space="PSUM") as pskvp:

            # scales broadcast to all partitions (tiny DMAs)
            sqa_sb = smallp.tile([P, 12, 16], F32, tag="sqa")
            nc.sync.dma_start(out=sqa_sb[:], in_=_bcast_ap(sqa))
            skva_sb = smallp.tile([P, 5, 16], F32, tag="skva")
            nc.sync.dma_start(out=skva_sb[:], in_=_bcast_ap(skva))
            qlnw_sb = smallp.tile([P, Q_LORA], F32, tag="qlnw")
            nc.sync.dma_start(out=qlnw_sb[:], in_=_bcast_ap(qlnw[0]))
            kvlnw_sb = smallp.tile([P, KV_LORA], F32, tag="kvlnw")
            nc.sync.dma_start(out=kvlnw_sb[:], in_=_bcast_ap(kvlnw[0]))
            cos_sb = smallp.tile([P, MT, ROPE], F32, tag="cos")
            nc.sync.dma_start(out=cos_sb[:], in_=cosr.rearrange("(m p) d -> p m d", p=P))
            sin_sb = smallp.tile([P, MT, ROPE], F32, tag="sin")
            nc.sync.dma_start(out=sin_sb[:], in_=sinr.rearrange("(m p) d -> p m d", p=P))

            eps_sb = smallp.tile([P, 1], F32, tag="eps")
            nc.vector.memset(eps_sb[:], EPS)

            x_sb = xp.tile([P, KT, R], F32R, tag="x")
            nc.sync.dma_start(out=x_sb[:], in_=xT.rearrange("(k p) r -> p k r", p=P))

            # weights: one tile per contraction block so DMA/dequant/matmul
            # dependencies stay tile-granular (whole-tensor deps serialize)
            wqa_t = [wqp.tile([P, Q_LORA], F32R, tag=f"wqa{k}", name=f"wqa{k}")
                     for k in range(KT)]
            wkva_t = [wkvp.tile([P, KV_LORA + ROPE], F32R, tag=f"wkva{k}",
                                name=f"wkva{k}") for k in range(KT)]
            for k in range(KT):
                nc.sync.dma_start(out=wqa_t[k][:], in_=wqaT[ts(k, P), :])
                nc.sync.dma_start(out=wkva_t[k][:], in_=wkvaT[ts(k, P), :])
                # wq_a^T tile k: [128, 1536] -> [128, 12, 128] * sq_a[j, k]
                w3 = wqa_t[k][:].rearrange("p (j n) -> p j n", n=P)
                nc.vector.tensor_mul(
                    w3, w3,
                    sqa_sb[:, :, k].unsqueeze(2).to_broadcast((P, 12, P)))
                wk = wkva_t[k][:, 0:KV_LORA].rearrange("p (j n) -> p j n", n=P)
                nc.vector.tensor_mul(
                    wk, wk,
                    skva_sb[:, 0:4, k].unsqueeze(2).to_broadcast((P, 4, P)))
                nc.vector.tensor_scalar_mul(
                    wkva_t[k][:, KV_LORA:],
                    wkva_t[k][:, KV_LORA:],
                    skva_sb[:, 4, k:k + 1])

            for m in range(MT):
                psq = psqp.tile([P, Q_LORA], F32, tag="psq")       # 3 banks
                pskv = pskvp.tile([P, KV_LORA + ROPE], F32, tag="pskv")  # 2 banks
                for k in range(KT):
                    lhs = x_sb[:, k, ts(m, P)]
                    for n in range(Q_LORA // SQB):
                        nc.tensor.matmul(psq[:, ts(n, SQB)], lhs,
                                         wqa_t[k][:, ts(n, SQB)],
                                         start=(k == 0), stop=(k == KT - 1))
                    nc.tensor.matmul(pskv[:, 0:KV_LORA], lhs,
                                     wkva_t[k][:, 0:KV_LORA],
                                     start=(k == 0), stop=(k == KT - 1))
                    nc.tensor.matmul(pskv[:, KV_LORA:], lhs,
                                     wkva_t[k][:, KV_LORA:],
                                     start=(k == 0), stop=(k == KT - 1))

                # rmsnorm(q): E[x^2] via Square-activation accumulate
                sq_scr = scrp.tile([P, Q_LORA], F32, tag="scr")
                ssq = statp.tile([P, 1], F32, tag="ssq")
                nc.scalar.activation(sq_scr[:], psq[:], AF.Square, accum_out=ssq[:])
                rms = statp.tile([P, 1], F32, tag="rms")
                nc.scalar.activation(rms[:], ssq[:], AF.Sqrt,
                                     scale=1.0 / Q_LORA, bias=eps_sb[:, 0:1])
                rinv = statp.tile([P, 1], F32, tag="rinv")
                nc.vector.reciprocal(rinv[:], rms[:])
                lnq_sb = outp.tile([P, Q_LORA], F32, tag="lnq")
                nc.vector.scalar_tensor_tensor(
                    lnq_sb[:], psq[:], rinv[:, 0:1], qlnw_sb[:],
                    op0=ALU.mult, op1=ALU.mult)
                nc.sync.dma_start(out=lnq[ts(m, P), :], in_=lnq_sb[:])

                # rmsnorm(kv)
                kv_scr = scrp.tile([P, KV_LORA], F32, tag="kscr")
                ssk = statp.tile([P, 1], F32, tag="ssk")
                nc.scalar.activation(kv_scr[:], pskv[:, 0:KV_LORA], AF.Square,
                                     accum_out=ssk[:])
                rmsk = statp.tile([P, 1], F32, tag="rmsk")
                nc.scalar.activation(rmsk[:], ssk[:], AF.Sqrt,
                                     scale=1.0 / KV_LORA, bias=eps_sb[:, 0:1])
                rinvk = statp.tile([P, 1], F32, tag="rinvk")
                nc.vector.reciprocal(rinvk[:], rmsk[:])
                lnkv_sb = outp.tile([P, KV_LORA], F32, tag="lnkv")
                nc.vector.scalar_tensor_tensor(
                    lnkv_sb[:], pskv[:, 0:KV_LORA], rinvk[:, 0:1], kvlnw_sb[:],
                    op0=ALU.mult, op1=ALU.mult)
                nc.sync.dma_start(out=lnkv[ts(m, P), :], in_=lnkv_sb[:])

                # rope on k_pe (natural layout: halves are column slices)
                H2 = ROPE // 2
                a = pskv[:, KV_LORA:KV_LORA + H2]
                b = pskv[:, KV_LORA + H2:]
                kpe_sb = outp.tile([P, ROPE], F32, tag="kpe")
                t1 = statp.tile([P, H2], F32, tag="t1")
                t2 = statp.tile([P, H2], F32, tag="t2")
                nc.vector.tensor_mul(t1[:], a, cos_sb[:, m, 0:H2])
                nc.vector.tensor_mul(t2[:], b, sin_sb[:, m, 0:H2])
                nc.vector.scalar_tensor_tensor(
                    kpe_sb[:, 0:H2], t2[:], -1.0, t1[:],
                    op0=ALU.mult, op1=ALU.add)
                t3 = statp.tile([P, H2], F32, tag="t3")
                t4 = statp.tile([P, H2], F32, tag="t4")
                nc.vector.tensor_mul(t3[:], b, cos_sb[:, m, H2:])
                nc.vector.tensor_mul(t4[:], a, sin_sb[:, m, H2:])
                nc.vector.scalar_tensor_tensor(
                    kpe_sb[:, H2:], t4[:], 1.0, t3[:],
                    op0=ALU.mult, op1=ALU.add)
                nc.sync.dma_start(out=kpe[ts(m, P), :], in_=kpe_sb[:])
    nc.compile()
    return nc


# --------------------------------------------------------------------------
# Launch 2: head-sharded b-projections + attention + o_proj partial
# --------------------------------------------------------------------------

def build_l2(reps=1, phases=("qb", "rope", "kv", "attn", "oproj")):
    nc = bacc.Bacc("TRN2", debug=False, num_devices=NC_N)
    lnqT = nc.dram_tensor("lnqT", [Q_LORA, S], F32R, kind="ExternalInput").ap()
    lnkvT = nc.dram_tensor("lnkvT", [KV_LORA, S], F32R, kind="ExternalInput").ap()
    kpeT = nc.dram_tensor("kpeT", [ROPE, S], F32R, kind="ExternalInput").ap()
    cosT = nc.dram_tensor("cosT", [ROPE, S], F32, kind="ExternalInput").ap()
    sinT = nc.dram_tensor("sinT", [ROPE, S], F32, kind="ExternalInput").ap()
    wqbT = nc.dram_tensor("wqbT", [Q_LORA, 3 * P], F32R, kind="ExternalInput").ap()
    sqbr = nc.dram_tensor("sqbr", [5, 12], F32, kind="ExternalInput").ap()
    wkvbT = nc.dram_tensor("wkvbT", [KV_LORA, 4 * P], F32R, kind="ExternalInput").ap()
    skvbr = nc.dram_tensor("skvbr", [4, 4], F32, kind="ExternalInput").ap()
    woT = nc.dram_tensor("woT", [HPC * VDIM, HID], F32R, kind="ExternalInput").ap()
    sor = nc.dram_tensor("sor", [16, 2], F32, kind="ExternalInput").ap()
    ones = nc.dram_tensor("ones", [P, P], F32R, kind="ExternalInput").ap()
    outT = nc.dram_tensor("outT", [HID, S], F32, kind="ExternalOutput").ap()

    H2 = ROPE // 2
    WQB_RUNS = [(0, 128), (128, 192), (192, 256), (256, 320), (320, 384)]

    with tile.TileContext(nc) as tc:
      for _rep in range(reps):
        with tc.tile_pool(name="pp", bufs=1) as pp, \
             tc.tile_pool(name="smallp", bufs=1) as smallp:

            # tiny run-scale tables, broadcast to all partitions
            sqbr_sb = smallp.tile([P, 5, 12], F32, tag="sqbr")
            nc.sync.dma_start(out=sqbr_sb[:], in_=_bcast_ap(sqbr))
            skvbr_sb = smallp.tile([P, 4, 4], F32, tag="skvbr")
            nc.sync.dma_start(out=skvbr_sb[:], in_=_bcast_ap(skvbr))
            sor_sb = smallp.tile([P, 16, 2], F32, tag="sor")
            nc.sync.dma_start(out=sor_sb[:], in_=_bcast_ap(sor))

            ones_sb = pp.tile([P, P], F32R, tag="ones")
            nc.sync.dma_start(out=ones_sb[:], in_=ones)

            # wide causal mask: maskw[r, c] = 1 iff c >= r + 384.
            # diagonal-offset d tile = maskw[:, 384-128d : 896-128d]
            maskw = pp.tile([P, 896], F32R, tag="maskw")
            nc.gpsimd.affine_select(
                out=maskw[:], in_=ones_sb[:, 0:1].to_broadcast((P, 896)),
                pattern=[[1, 896]], compare_op=ALU.is_ge,
                fill=0.0, base=-384, channel_multiplier=-1)

            # o_proj weights resident (needed at the end)
            wo_t = [pp.tile([P, HID], F32R, tag=f"wo{k}", name=f"wo{k}")
                    for k in range(2)]
            for k in range(2):
                nc.sync.dma_start(out=wo_t[k][:], in_=woT[ts(k, P), :])
                for j in range(16):
                    nc.vector.tensor_scalar_mul(
                        wo_t[k][:, ts(j, P)], wo_t[k][:, ts(j, P)],
                        sor_sb[:, j, k:k + 1])

            # k_pe duplicated onto both partition halves (heads 0/1 alignment)
            kpe2_sb = pp.tile([P, S], F32R, tag="kpe2")
            nc.sync.dma_start(out=kpe2_sb[0:ROPE, :], in_=kpeT)
            nc.sync.dma_start(out=kpe2_sb[ROPE:, :], in_=kpeT)

            qn = [[pp.tile([P, 1024], F32R, tag=f"qn{h}_{hf}",
                           name=f"qn{h}_{hf}") for hf in range(2)]
                  for h in range(HPC)]
            qpe_all = pp.tile([P, S], F32R, tag="qpe")  # rows 0:64 h0, 64:128 h1
            kn = [[pp.tile([P, SQB], F32R, tag=f"kn{h}_{sq}",
                           name=f"kn{h}_{sq}") for sq in range(NSQB)]
                  for h in range(HPC)]
            v_t = [pp.tile([P, HPC * VDIM], F32R, tag=f"v{t}", name=f"v{t}")
                   for t in range(NSKT)]
            attnT = [pp.tile([P, S], F32R, tag=f"at{h}", name=f"at{h}")
                     for h in range(HPC)]

            # kv-phase inputs: declare + DMA early so loads prefetch
            # during the q_b phase (pool scoping would otherwise delay them)
            lnkv_t = [pp.tile([P, S], F32R, tag=f"lnkv{k}",
                              name=f"lnkv{k}") for k in range(4)]
            wkvb_t = [pp.tile([P, 4 * P], F32R, tag=f"wkvb{k}",
                              name=f"wkvb{k}") for k in range(4)]
            for k in range(4):
                nc.scalar.dma_start(out=wkvb_t[k][:], in_=wkvbT[ts(k, P), :])
                nc.scalar.dma_start(out=lnkv_t[k][:], in_=lnkvT[ts(k, P), :])
                for r in range(4):
                    nc.vector.tensor_scalar_mul(
                        wkvb_t[k][:, ts(r, P)], wkvb_t[k][:, ts(r, P)],
                        skvbr_sb[:, r, k:k + 1])

            # ---------- q_b projection (streamed over lnqT) ----------
            KQ = Q_LORA // P  # 12
            if "qb" in phases:
              with tc.tile_pool(name="wqbp", bufs=1) as wqbp, \
                 tc.tile_pool(name="lnqsp", bufs=8) as lnqsp, \
                 tc.tile_pool(name="psqb", bufs=1, space="PSUM") as psqb:
                wqb_t = [wqbp.tile([P, 3 * P], F32R, tag=f"wqb{k}",
                                   name=f"wqb{k}") for k in range(KQ)]
                for k in range(KQ):
                    nc.sync.dma_start(out=wqb_t[k][:], in_=wqbT[ts(k, P), :])
                    for r, (a, b) in enumerate(WQB_RUNS):
                        nc.vector.tensor_scalar_mul(
                            wqb_t[k][:, a:b], wqb_t[k][:, a:b],
                            sqbr_sb[:, r, k:k + 1])
                for hf in range(2):
                    ps_mo = [psqb.tile([P, 1024], F32, tag=f"qb{mo}",
                                       name=f"psqb{mo}") for mo in range(3)]
                    for k in range(KQ):
                        lt = lnqsp.tile([P, 1024], F32R, tag="lnqs")
                        nc.sync.dma_start(out=lt[:],
                                           in_=lnqT[ts(k, P), ts(hf, 1024)])
                        for mo in range(3):
                            for sq in range(2):
                                nc.tensor.matmul(
                                    ps_mo[mo][:, ts(sq, SQB)],
                                    wqb_t[k][:, ts(mo, P)],
                                    lt[:, ts(sq, SQB)],
                                    start=(k == 0), stop=(k == KQ - 1))
                    for h in range(HPC):
                        nc.vector.tensor_copy(qn[h][hf][:], ps_mo[h][:])
                    nc.vector.tensor_copy(qpe_all[:, ts(hf, 1024)], ps_mo[2][:])

            # ---------- rope on q_pe ----------
            if "rope" in phases:
              with tc.tile_pool(name="ropep", bufs=1) as rp:
                cos2_sb = rp.tile([P, S], F32, tag="cos2")
                nc.sync.dma_start(out=cos2_sb[0:ROPE, :], in_=cosT)
                nc.sync.dma_start(out=cos2_sb[ROPE:, :], in_=cosT)
                # sign-adjusted sin (multiplies the swapped-half operand)
                sing2_sb = rp.tile([P, S], F32, tag="sing2")
                nc.sync.dma_start(out=sing2_sb[0:ROPE, :], in_=sinT)
                nc.sync.dma_start(out=sing2_sb[ROPE:, :], in_=sinT)
                nc.vector.tensor_scalar_mul(sing2_sb[0:H2, :],
                                            sing2_sb[0:H2, :], -1.0)
                nc.vector.tensor_scalar_mul(sing2_sb[ROPE:ROPE + H2, :],
                                            sing2_sb[ROPE:ROPE + H2, :], -1.0)
                qsw = rp.tile([P, S], F32R, tag="qsw")
                for h in range(HPC):
                    o = h * ROPE
                    nc.sync.dma_start(out=qsw[o:o + H2, :],
                                      in_=qpe_all[o + H2:o + ROPE, :])
                    nc.sync.dma_start(out=qsw[o + H2:o + ROPE, :],
                                      in_=qpe_all[o:o + H2, :])
                rt = rp.tile([P, S], F32, tag="ropet")
                nc.vector.tensor_mul(rt[:], qpe_all[:], cos2_sb[:])
                ru = rp.tile([P, S], F32, tag="ropeu")
                nc.vector.tensor_mul(ru[:], qsw[:], sing2_sb[:])
                nc.vector.tensor_add(qpe_all[:], rt[:], ru[:])

            # ---------- kv_b projection ----------
            if "kv" in phases:
              with tc.tile_pool(name="pskv", bufs=2, space="PSUM") as pskvp:
                # interleaved by sq chunk so attention block b can start as
                # soon as its kn/v tiles land (per-block tiles keep deps tight)
                for sq in range(NSQB):
                    for h in range(HPC):
                        ps = pskvp.tile([P, SQB], F32, tag="pskn")
                        for k in range(4):
                            nc.tensor.matmul(ps[:], wkvb_t[k][:, ts(h, P)],
                                             lnkv_t[k][:, ts(sq, SQB)],
                                             start=(k == 0), stop=(k == 3))
                        nc.vector.tensor_copy(kn[h][sq][:], ps[:])
                    for t in range(4 * sq, 4 * sq + 4):
                        ps = pskvp.tile([P, HPC * VDIM], F32, tag="psv")
                        for k in range(4):
                            nc.tensor.matmul(ps[:], lnkv_t[k][:, ts(t, P)],
                                             wkvb_t[k][:, 2 * P:4 * P],
                                             start=(k == 0), stop=(k == 3))
                        nc.vector.tensor_copy(v_t[t][:], ps[:])

            # ---------- attention ----------
            if "attn" in phases:
              with tc.tile_pool(name="probsp", bufs=8) as probsp, \
                 tc.tile_pool(name="sumsp", bufs=4) as sumsp, \
                 tc.tile_pool(name="recp", bufs=4) as recp, \
                 tc.tile_pool(name="pscp", bufs=3, space="PSUM") as pscp, \
                 tc.tile_pool(name="patp", bufs=2, space="PSUM") as patp:
                for h in range(HPC):
                    o = h * ROPE
                    for b in range(NSQB):
                        nsk = 4 * (b + 1)
                        ps_at = patp.tile([P, SQB], F32, tag="psat")
                        sumacc = sumsp.tile([P, SQB], F32R, tag="sumacc")
                        for t in range(nsk):
                            ps_s = pscp.tile([P, SQB], F32, tag="pss", bufs=5)
                            nc.tensor.matmul(
                                ps_s[:], kn[h][t // 4][:, ts(t % 4, P)],
                                qn[h][b // 2][:, ts(b % 2, SQB)],
                                start=True, stop=False)
                            nc.tensor.matmul(ps_s[:],
                                             kpe2_sb[o:o + ROPE, ts(t, P)],
                                             qpe_all[o:o + ROPE, ts(b, SQB)],
                                             start=False, stop=True)
                            pt = probsp.tile([P, SQB], F32R, tag="probs")
                            nc.scalar.activation(pt[:], ps_s[:], AF.Exp,
                                                 bias=0.0, scale=SOFTMAX_SCALE)
                            d = t - 4 * b
                            if d >= 0:
                                nc.gpsimd.tensor_mul(
                                    pt[:], pt[:],
                                    maskw[:, 384 - 128 * d:896 - 128 * d])
                            if t == 0:
                                nc.vector.tensor_copy(sumacc[:], pt[:])
                            else:
                                nc.vector.tensor_add(sumacc[:], sumacc[:], pt[:])
                            nc.tensor.matmul(ps_at[:], v_t[t][:, ts(h, VDIM)],
                                             pt[:],
                                             start=(t == 0), stop=(t == nsk - 1))
                        ps_sum = pscp.tile([P, SQB], F32, tag="pssum", bufs=1)
                        nc.tensor.matmul(ps_sum[:], ones_sb[:], sumacc[:],
                                         start=True, stop=True)
                        rec = recp.tile([P, SQB], F32, tag="rec")
                        nc.vector.reciprocal(rec[:], ps_sum[:])
                        nc.vector.tensor_mul(attnT[h][:, ts(b, SQB)],
                                             ps_at[:], rec[:])

            # ---------- o_proj partial: outT[o, s] = sum_pc wo[o,pc] attnT[pc,s]
            if "oproj" in phases:
              with tc.tile_pool(name="ostp", bufs=3) as ostp, \
                 tc.tile_pool(name="psop", bufs=2, space="PSUM") as psop:
                for mo in range(HID // P):
                    po = psop.tile([P, S], F32, tag="pso")
                    for k in range(HPC):
                        for sq in range(NSQB):
                            nc.tensor.matmul(po[:, ts(sq, SQB)],
                                             wo_t[k][:, ts(mo, P)],
                                             attnT[k][:, ts(sq, SQB)],
                                             start=(k == 0), stop=(k == HPC - 1))
                    ost = ostp.tile([P, S], F32, tag="ost")
                    for sq in range(NSQB):
                        if sq == 3:
                            nc.scalar.copy(ost[:, ts(sq, SQB)], po[:, ts(sq, SQB)])
                        else:
                            nc.vector.tensor_copy(ost[:, ts(sq, SQB)],
                                                  po[:, ts(sq, SQB)])
                    for dq in range(4):
                        nc.sync.dma_start(out=outT[ts(mo, P), ds(dq * 512, 512)],
                                          in_=ost[:, ts(dq, SQB)])
    nc.compile()
    return nc


# --------------------------------------------------------------------------
# Host orchestration
# --------------------------------------------------------------------------

_CACHE = {}
_LAST_L1_MAPS = None
_LAST_L2_MAPS = None


def _get(name, builder):
    if name not in _CACHE:
        _CACHE[name] = builder()
    return _CACHE[name]


class _SimResults:
    def __init__(self, results):
        self.results = results
        self.exec_time_ns = None


def _run(nc, in_maps, core_ids):
    if os.environ.get("BASS_KERNEL_SIM"):
        from concourse.bass_interp import CoreSim
        results = []
        out_names = [
            alloc.memorylocations[0].name
            for alloc in nc.m.functions[0].allocations
            if getattr(alloc, "kind", None) == "ExternalOutput"
            and getattr(alloc, "memorylocations", None)
        ]
        for in_map in in_maps:
            sim = CoreSim(nc, trace=False)
            for k, v in in_map.items():
                sim.tensor(k)[:] = v
            sim.simulate(check_with_hw=False)
            results.append({n: np.array(sim.tensor(n)) for n in out_names})
        return _SimResults(results)
    return run_bass_kernel_spmd(nc, in_maps, core_ids=core_ids)


def _c(a):
    return np.ascontiguousarray(a, dtype=np.float32)


def run_l1(hidden_states, wq_a, sq_a, wkv_a, skv_a, q_ln_w, kv_ln_w, cos, sin):
    nc = _get("l1", build_l1)
    wqaT = _c(wq_a.T)
    wkvaT = _c(wkv_a.T)
    in_maps = []
    for c in range(NC_N):
        rows = slice(c * R, (c + 1) * R)
        in_maps.append({
            "xT": _c(hidden_states[rows].T),
            "wqaT": wqaT,
            "wkvaT": wkvaT,
            "sqa": _c(sq_a),
            "skva": _c(skv_a),
            "qlnw": _c(q_ln_w[None, :]),
            "kvlnw": _c(kv_ln_w[None, :]),
            "cosr": _c(cos[rows]),
            "sinr": _c(sin[rows]),
        })
    global _LAST_L1_MAPS
    _LAST_L1_MAPS = in_maps
    res = _run(nc, in_maps, list(range(NC_N)))
    lnq = np.concatenate([r["lnq"] for r in res.results], axis=0)
    lnkv = np.concatenate([r["lnkv"] for r in res.results], axis=0)
    kpe = np.concatenate([r["kpe"] for r in res.results], axis=0)
    return lnq, lnkv, kpe


def _l2_weight_shards(c, wq_b, sq_b, wkv_b, skv_b, wo, so):
    h0, h1 = HPC * c, HPC * c + 1
    # wq_b rows reordered [nope_h0 | nope_h1 | pe_h0 | pe_h1]
    wqb = wq_b.reshape(NH, HEAD, Q_LORA)
    rows = np.concatenate([
        np.arange(h0 * HEAD, h0 * HEAD + NOPE),
        np.arange(h1 * HEAD, h1 * HEAD + NOPE),
        np.arange(h0 * HEAD + NOPE, (h0 + 1) * HEAD),
        np.arange(h1 * HEAD + NOPE, (h1 + 1) * HEAD),
    ])
    wqbT = _c(wq_b[rows].T)                      # [1536, 384]
    # run-constant scale table: runs [0:128,128:192,192:256,256:320,320:384]
    # hit original row-blocks [3c, 3c+1, 3c+2, 3c+1, 3c+2]
    run_blk = [3 * c, 3 * c + 1, 3 * c + 2, 3 * c + 1, 3 * c + 2]
    sqbr = _c(sq_b[run_blk, :])                  # [5, 12]

    # wkv_b rows reordered [kn_h0 | kn_h1 | v_h0 | v_h1]
    krows = np.concatenate([
        np.arange(h0 * (NOPE + VDIM), h0 * (NOPE + VDIM) + NOPE),
        np.arange(h1 * (NOPE + VDIM), h1 * (NOPE + VDIM) + NOPE),
        np.arange(h0 * (NOPE + VDIM) + NOPE, (h0 + 1) * (NOPE + VDIM)),
        np.arange(h1 * (NOPE + VDIM) + NOPE, (h1 + 1) * (NOPE + VDIM)),
    ])
    wkvbT = _c(wkv_b[krows].T)                   # [512, 512]
    # runs of 128 hit original row-blocks [4c, 4c+2, 4c+1, 4c+3]
    kv_run_blk = [4 * c, 4 * c + 2, 4 * c + 1, 4 * c + 3]
    skvbr = _c(skv_b[kv_run_blk, :])             # [4, 4]

    cols = np.concatenate([np.arange(h0 * VDIM, (h0 + 1) * VDIM),
                           np.arange(h1 * VDIM, (h1 + 1) * VDIM)])
    woT = _c(wo[:, cols].T)                      # [256, 2048]
    # sor[j, kk] = so[out-block j, in-block of head kk]
    sor = _c(so[:, [2 * c, 2 * c + 1]])          # [16, 2]
    return wqbT, sqbr, wkvbT, skvbr, woT, sor


def run_l2(lnq, lnkv, kpe, cos, sin, wq_b, sq_b, wkv_b, skv_b, wo, so):
    nc = _get("l2", build_l2)
    lnqT = _c(lnq.T)
    lnkvT = _c(lnkv.T)
    kpeT = _c(kpe.T)
    cosT = _c(cos.T)
    sinT = _c(sin.T)
    ones = np.ones((P, P), dtype=np.float32)
    in_maps = []
    for c in range(NC_N):
        wqbT, sqbr, wkvbT, skvbr, woT, sor = _l2_weight_shards(
            c, wq_b, sq_b, wkv_b, skv_b, wo, so)
        in_maps.append({
            "lnqT": lnqT, "lnkvT": lnkvT, "kpeT": kpeT,
            "cosT": cosT, "sinT": sinT,
            "wqbT": wqbT, "sqbr": sqbr,
            "wkvbT": wkvbT, "skvbr": skvbr,
            "woT": woT, "sor": sor,
            "ones": ones,
        })
    global _LAST_L2_MAPS
    _LAST_L2_MAPS = in_maps
    res = _run(nc, in_maps, list(range(NC_N)))
    acc = res.results[0]["outT"].astype(np.float32)
    for c in range(1, NC_N):
        acc = acc + res.results[c]["outT"]
    return _c(acc.T)


def kernel(hidden_states, cos, sin, wq_a, sq_a, wq_b, sq_b, wkv_a, skv_a,
           wkv_b, skv_b, wo, so, q_ln_w, kv_ln_w):
    lnq, lnkv, kpe = run_l1(hidden_states, wq_a, sq_a, wkv_a, skv_a,
                            q_ln_w, kv_ln_w, cos, sin)
    return run_l2(lnq, lnkv, kpe, cos, sin, wq_b, sq_b, wkv_b, skv_b, wo, so)



# revision 4
# speedup vs baseline: 1.1665x; 1.1665x over previous
"""DeepseekV3 MLA prefill attention on 8 Trainium2 NeuronCores.

Strategy (no on-device collectives; host does shard/gather data movement only):

  Launch 1 (row-sharded): each core takes S/8=256 sequence rows and computes the
    low-rank "a" projections for its rows: lq = x @ dq(wq_a).T -> rmsnorm,
    lkv = x @ dq(wkv_a).T -> rmsnorm(kv part) + rope(k_pe part).
    Weights wq_a/wkv_a are replicated (reading them once per core is unavoidable
    without cross-core comms; compute is small).

  Host: gathers the 8 row-shards, transposes to feature-major layout.

  Launch 2 (head-sharded, 2 of 16 heads per core): q_b / kv_b projections for the
    core's heads (column-parallel), causal attention in transposed layout
    (scores^T = K^T-tiles x Q^T, softmax without max-subtraction -- scores are
    provably small for this distribution -- unnormalized probs, PV accumulation,
    normalization by matmul-computed column sums), then the o_proj row-parallel
    partial product for the core's 256 input columns.

  Host: sums the 8 o_proj partials (the all-reduce of the sharding hint, done at
  the gather step) and transposes back to [S, HID].

Data plane is bf16 (f32 PSUM accumulation): halves HBM traffic and doubles DVE
throughput; matmul rate on the PE is the same as full-rate fp32. Dequantization
of the block-scaled weights happens on device via broadcast-AP tensor ops.
"""

import math
import os

import numpy as np
import ml_dtypes

import concourse.bass as bass
import concourse.bacc as bacc
import concourse.mybir as mybir
import concourse.tile as tile
from concourse.bass import ts, ds
from concourse.bass_utils import run_bass_kernel_spmd

F32 = mybir.dt.float32
F32R = mybir.dt.float32r
BF = mybir.dt.bfloat16
AF = mybir.ActivationFunctionType
ALU = mybir.AluOpType

S, HID = 2048, 2048
NH, Q_LORA, KV_LORA = 16, 1536, 512
NOPE, ROPE, VDIM = 128, 64, 128
HEAD = NOPE + ROPE            # 192
NC_N = 8                      # cores
HPC = NH // NC_N              # heads per core = 2
R = S // NC_N                 # rows per core in L1 = 256
EPS = 1e-6
MSCALE = 0.1 * 1.0 * math.log(40.0) + 1.0
SOFTMAX_SCALE = HEAD ** -0.5 * MSCALE * MSCALE

P = 128
SQB = 512                     # q-column block in attention
NSQB = S // SQB               # 4
NSKT = S // P                 # 16 sk tiles

BF_NP = ml_dtypes.bfloat16


def _bcast_ap(ap, p=P):
    """DRAM/SBUF AP broadcast across p partitions (step-0 partition dim)."""
    return bass.AP(tensor=ap.tensor, offset=ap.offset, ap=[[0, p]] + list(ap.ap))


# --------------------------------------------------------------------------
# Launch 1: row-sharded a-projections + rmsnorm + k_pe rope
# --------------------------------------------------------------------------

def build_l1(reps=1):
    nc = bacc.Bacc("TRN2", debug=False, num_devices=NC_N)
    xT = nc.dram_tensor("xT", [HID, R], BF, kind="ExternalInput").ap()
    wqaT = nc.dram_tensor("wqaT", [HID, Q_LORA], BF, kind="ExternalInput").ap()
    wkvaT = nc.dram_tensor("wkvaT", [HID, KV_LORA + ROPE], BF, kind="ExternalInput").ap()
    sqa = nc.dram_tensor("sqa", [12, 16], F32, kind="ExternalInput").ap()
    skva = nc.dram_tensor("skva", [5, 16], F32, kind="ExternalInput").ap()
    qlnw = nc.dram_tensor("qlnw", [1, Q_LORA], F32, kind="ExternalInput").ap()
    kvlnw = nc.dram_tensor("kvlnw", [1, KV_LORA], F32, kind="ExternalInput").ap()
    cosr = nc.dram_tensor("cosr", [R, ROPE], F32, kind="ExternalInput").ap()
    sinr = nc.dram_tensor("sinr", [R, ROPE], F32, kind="ExternalInput").ap()
    lnq = nc.dram_tensor("lnq", [R, Q_LORA], BF, kind="ExternalOutput").ap()
    lnkv = nc.dram_tensor("lnkv", [R, KV_LORA], BF, kind="ExternalOutput").ap()
    kpe = nc.dram_tensor("kpe", [R, ROPE], BF, kind="ExternalOutput").ap()

    KT = HID // P   # 16 contraction tiles
    MT = R // P     # 2 row tiles

    with tile.TileContext(nc) as tc:
      for _rep in range(reps):
        with tc.tile_pool(name="wq", bufs=1) as wqp, \
             tc.tile_pool(name="wkv", bufs=1) as wkvp, \
             tc.tile_pool(name="xp", bufs=1) as xp, \
             tc.tile_pool(name="small", bufs=1) as smallp, \
             tc.tile_pool(name="stat", bufs=8) as statp, \
             tc.tile_pool(name="scratch", bufs=2) as scrp, \
             tc.tile_pool(name="outp", bufs=4) as outp, \
             tc.tile_pool(name="psq", bufs=2, space="PSUM") as psqp, \
             tc.tile_pool(name="pskv", bufs=1, space="PSUM") as pskvp:

            # scales broadcast to all partitions (tiny DMAs)
            sqa_sb = smallp.tile([P, 12, 16], F32, tag="sqa")
            nc.sync.dma_start(out=sqa_sb[:], in_=_bcast_ap(sqa))
            skva_sb = smallp.tile([P, 5, 16], F32, tag="skva")
            nc.sync.dma_start(out=skva_sb[:], in_=_bcast_ap(skva))
            qlnw_sb = smallp.tile([P, Q_LORA], F32, tag="qlnw")
            nc.scalar.dma_start(out=qlnw_sb[:], in_=_bcast_ap(qlnw[0]))
            kvlnw_sb = smallp.tile([P, KV_LORA], F32, tag="kvlnw")
            nc.scalar.dma_start(out=kvlnw_sb[:], in_=_bcast_ap(kvlnw[0]))
            cos_sb = smallp.tile([P, MT, ROPE], F32, tag="cos")
            nc.scalar.dma_start(out=cos_sb[:], in_=cosr.rearrange("(m p) d -> p m d", p=P))
            sin_sb = smallp.tile([P, MT, ROPE], F32, tag="sin")
            nc.scalar.dma_start(out=sin_sb[:], in_=sinr.rearrange("(m p) d -> p m d", p=P))

            eps_sb = smallp.tile([P, 1], F32, tag="eps")
            nc.vector.memset(eps_sb[:], EPS)

            # x on the scalar queue so it doesn't delay the weight stream
            x_sb = xp.tile([P, KT, R], BF, tag="x")
            nc.scalar.dma_start(out=x_sb[:], in_=xT.rearrange("(k p) r -> p k r", p=P))

            # weights: one tile per contraction block so DMA/dequant/matmul
            # dependencies stay tile-granular (whole-tensor deps serialize)
            wqa_t = [wqp.tile([P, Q_LORA], BF, tag=f"wqa{k}", name=f"wqa{k}")
                     for k in range(KT)]
            wkva_t = [wkvp.tile([P, KV_LORA + ROPE], BF, tag=f"wkva{k}",
                                name=f"wkva{k}") for k in range(KT)]
            for k in range(KT):
                nc.sync.dma_start(out=wqa_t[k][:], in_=wqaT[ts(k, P), :])
                nc.sync.dma_start(out=wkva_t[k][:], in_=wkvaT[ts(k, P), :])
                # wq_a^T tile k: [128, 1536] -> [128, 12, 128] * sq_a[j, k]
                w3 = wqa_t[k][:].rearrange("p (j n) -> p j n", n=P)
                nc.vector.tensor_mul(
                    w3, w3,
                    sqa_sb[:, :, k].unsqueeze(2).to_broadcast((P, 12, P)))
                wk = wkva_t[k][:, 0:KV_LORA].rearrange("p (j n) -> p j n", n=P)
                nc.gpsimd.tensor_mul(
                    wk, wk,
                    skva_sb[:, 0:4, k].unsqueeze(2).to_broadcast((P, 4, P)))
                nc.gpsimd.tensor_mul(
                    wkva_t[k][:, KV_LORA:],
                    wkva_t[k][:, KV_LORA:],
                    skva_sb[:, 4, k:k + 1].to_broadcast((P, ROPE)))

            for m in range(MT):
                psq = psqp.tile([P, Q_LORA], F32, tag="psq")       # 3 banks
                pskv = pskvp.tile([P, KV_LORA + ROPE], F32, tag="pskv")  # 2 banks
                for k in range(KT):
                    lhs = x_sb[:, k, ts(m, P)]
                    for n in range(Q_LORA // SQB):
                        nc.tensor.matmul(psq[:, ts(n, SQB)], lhs,
                                         wqa_t[k][:, ts(n, SQB)],
                                         start=(k == 0), stop=(k == KT - 1))
                    nc.tensor.matmul(pskv[:, 0:KV_LORA], lhs,
                                     wkva_t[k][:, 0:KV_LORA],
                                     start=(k == 0), stop=(k == KT - 1))
                    nc.tensor.matmul(pskv[:, KV_LORA:], lhs,
                                     wkva_t[k][:, KV_LORA:],
                                     start=(k == 0), stop=(k == KT - 1))

                # rmsnorm(kv) first: its output DMA + rope overlap the q-norm
                kv_scr = scrp.tile([P, KV_LORA], F32, tag="kscr")
                ssk = statp.tile([P, 1], F32, tag="ssk")
                nc.scalar.activation(kv_scr[:], pskv[:, 0:KV_LORA], AF.Square,
                                     accum_out=ssk[:])
                rmsk = statp.tile([P, 1], F32, tag="rmsk")
                nc.scalar.activation(rmsk[:], ssk[:], AF.Sqrt,
                                     scale=1.0 / KV_LORA, bias=eps_sb[:, 0:1])
                rinvk = statp.tile([P, 1], F32, tag="rinvk")
                nc.vector.reciprocal(rinvk[:], rmsk[:])
                lnkv_sb = outp.tile([P, KV_LORA], BF, tag="lnkv")
                nc.vector.scalar_tensor_tensor(
                    lnkv_sb[:], pskv[:, 0:KV_LORA], rinvk[:, 0:1], kvlnw_sb[:],
                    op0=ALU.mult, op1=ALU.mult)
                nc.sync.dma_start(out=lnkv[ts(m, P), :], in_=lnkv_sb[:])

                # rope on k_pe (natural layout: halves are column slices)
                H2 = ROPE // 2
                a = pskv[:, KV_LORA:KV_LORA + H2]
                b = pskv[:, KV_LORA + H2:]
                kpe_sb = outp.tile([P, ROPE], BF, tag="kpe")
                t1 = statp.tile([P, H2], F32, tag="t1")
                t2 = statp.tile([P, H2], F32, tag="t2")
                nc.vector.tensor_mul(t1[:], a, cos_sb[:, m, 0:H2])
                nc.vector.tensor_mul(t2[:], b, sin_sb[:, m, 0:H2])
                nc.vector.scalar_tensor_tensor(
                    kpe_sb[:, 0:H2], t2[:], -1.0, t1[:],
                    op0=ALU.mult, op1=ALU.add)
                t3 = statp.tile([P, H2], F32, tag="t3")
                t4 = statp.tile([P, H2], F32, tag="t4")
                nc.vector.tensor_mul(t3[:], b, cos_sb[:, m, H2:])
                nc.vector.tensor_mul(t4[:], a, sin_sb[:, m, H2:])
                nc.vector.scalar_tensor_tensor(
                    kpe_sb[:, H2:], t4[:], 1.0, t3[:],
                    op0=ALU.mult, op1=ALU.add)
                nc.sync.dma_start(out=kpe[ts(m, P), :], in_=kpe_sb[:])

                # rmsnorm(q): E[x^2] via Square-activation accumulate
                sq_scr = scrp.tile([P, Q_LORA], F32, tag="scr")
                ssq = statp.tile([P, 1], F32, tag="ssq")
                nc.scalar.activation(sq_scr[:], psq[:], AF.Square, accum_out=ssq[:])
                rms = statp.tile([P, 1], F32, tag="rms")
                nc.scalar.activation(rms[:], ssq[:], AF.Sqrt,
                                     scale=1.0 / Q_LORA, bias=eps_sb[:, 0:1])
                rinv = statp.tile([P, 1], F32, tag="rinv")
                nc.vector.reciprocal(rinv[:], rms[:])
                # store lnq in 512-col chunks so DMA-out starts early
                for n in range(Q_LORA // SQB):
                    lnq_sb = outp.tile([P, SQB], BF, tag=f"lnq{n}",
                                       name=f"lnq{n}")
                    nc.vector.scalar_tensor_tensor(
                        lnq_sb[:], psq[:, ts(n, SQB)], rinv[:, 0:1],
                        qlnw_sb[:, ts(n, SQB)],
                        op0=ALU.mult, op1=ALU.mult)
                    nc.sync.dma_start(out=lnq[ts(m, P), ts(n, SQB)],
                                      in_=lnq_sb[:])
    nc.compile()
    return nc


# --------------------------------------------------------------------------
# Launch 2: head-sharded b-projections + attention + o_proj partial
# --------------------------------------------------------------------------

def build_l2(reps=1, phases=("qb", "rope", "kv", "attn", "oproj")):
    nc = bacc.Bacc("TRN2", debug=False, num_devices=NC_N)
    lnqT = nc.dram_tensor("lnqT", [Q_LORA, S], BF, kind="ExternalInput").ap()
    lnkvT = nc.dram_tensor("lnkvT", [KV_LORA, S], BF, kind="ExternalInput").ap()
    kpeT = nc.dram_tensor("kpeT", [ROPE, S], BF, kind="ExternalInput").ap()
    cosT = nc.dram_tensor("cosT", [ROPE, S], BF, kind="ExternalInput").ap()
    sinT = nc.dram_tensor("sinT", [ROPE, S], BF, kind="ExternalInput").ap()
    wqbT = nc.dram_tensor("wqbT", [Q_LORA, 3 * P], BF, kind="ExternalInput").ap()
    sqbr = nc.dram_tensor("sqbr", [5, 12], F32, kind="ExternalInput").ap()
    wkvbT = nc.dram_tensor("wkvbT", [KV_LORA, 4 * P], BF, kind="ExternalInput").ap()
    skvbr = nc.dram_tensor("skvbr", [4, 4], F32, kind="ExternalInput").ap()
    woT = nc.dram_tensor("woT", [HPC * VDIM, HID], BF, kind="ExternalInput").ap()
    sor = nc.dram_tensor("sor", [16, 2], F32, kind="ExternalInput").ap()
    ones = nc.dram_tensor("ones", [P, P], F32R, kind="ExternalInput").ap()
    outT = nc.dram_tensor("outT", [HID, S], BF, kind="ExternalOutput").ap()

    H2 = ROPE // 2
    WQB_RUNS = [(0, 128), (128, 192), (192, 256), (256, 320), (320, 384)]

    with tile.TileContext(nc) as tc:
      for _rep in range(reps):
        with tc.tile_pool(name="pp", bufs=1) as pp, \
             tc.tile_pool(name="smallp", bufs=1) as smallp:

            # tiny run-scale tables, broadcast to all partitions
            sqbr_sb = smallp.tile([P, 5, 12], F32, tag="sqbr")
            nc.sync.dma_start(out=sqbr_sb[:], in_=_bcast_ap(sqbr))
            skvbr_sb = smallp.tile([P, 4, 4], F32, tag="skvbr")
            nc.sync.dma_start(out=skvbr_sb[:], in_=_bcast_ap(skvbr))
            sor_sb = smallp.tile([P, 16, 2], F32, tag="sor")
            nc.sync.dma_start(out=sor_sb[:], in_=_bcast_ap(sor))

            ones_sb = pp.tile([P, P], F32R, tag="ones")
            nc.sync.dma_start(out=ones_sb[:], in_=ones)

            # wide causal mask: maskw[r, c] = 1 iff c >= r + 384.
            # diagonal-offset d tile = maskw[:, 384-128d : 896-128d]
            onesb_sb = pp.tile([P, 1], BF, tag="onesb")
            nc.vector.memset(onesb_sb[:], 1.0)
            maskw = pp.tile([P, 896], BF, tag="maskw")
            nc.gpsimd.affine_select(
                out=maskw[:], in_=onesb_sb[:, 0:1].to_broadcast((P, 896)),
                pattern=[[1, 896]], compare_op=ALU.is_ge,
                fill=0.0, base=-384, channel_multiplier=-1)

            # ---- q_b weight stream first: it gates the first matmuls ----
            KQ = Q_LORA // P  # 12
            wqb_t = [pp.tile([P, 3 * P], BF, tag=f"wqb{k}", name=f"wqb{k}")
                     for k in range(KQ)]
            for k in range(KQ):
                nc.sync.dma_start(out=wqb_t[k][:], in_=wqbT[ts(k, P), :])
                for r, (a, b) in enumerate(WQB_RUNS):
                    nc.vector.tensor_scalar_mul(
                        wqb_t[k][:, a:b], wqb_t[k][:, a:b],
                        sqbr_sb[:, r, k:k + 1])

            # rope tables + k_pe on the scalar queue (needed right after q_b)
            cos2_sb = pp.tile([P, S], BF, tag="cos2")
            nc.scalar.dma_start(out=cos2_sb[0:ROPE, :], in_=cosT)
            nc.scalar.dma_start(out=cos2_sb[ROPE:, :], in_=cosT)
            # sign-adjusted sin (multiplies the swapped-half operand)
            sing2_sb = pp.tile([P, S], BF, tag="sing2")
            nc.scalar.dma_start(out=sing2_sb[0:ROPE, :], in_=sinT)
            nc.scalar.dma_start(out=sing2_sb[ROPE:, :], in_=sinT)
            nc.vector.tensor_scalar_mul(sing2_sb[0:H2, :],
                                        sing2_sb[0:H2, :], -1.0)
            nc.vector.tensor_scalar_mul(sing2_sb[ROPE:ROPE + H2, :],
                                        sing2_sb[ROPE:ROPE + H2, :], -1.0)

            # k_pe duplicated onto both partition halves (heads 0/1 alignment)
            kpe2_sb = pp.tile([P, S], BF, tag="kpe2")
            nc.scalar.dma_start(out=kpe2_sb[0:ROPE, :], in_=kpeT)
            nc.scalar.dma_start(out=kpe2_sb[ROPE:, :], in_=kpeT)

            # kv-phase inputs on the scalar queue (needed ~mid-kernel)
            lnkv_t = [pp.tile([P, S], BF, tag=f"lnkv{k}",
                              name=f"lnkv{k}") for k in range(4)]
            wkvb_t = [pp.tile([P, 4 * P], BF, tag=f"wkvb{k}",
                              name=f"wkvb{k}") for k in range(4)]
            for k in range(4):
                nc.scalar.dma_start(out=wkvb_t[k][:], in_=wkvbT[ts(k, P), :])
                nc.scalar.dma_start(out=lnkv_t[k][:], in_=lnkvT[ts(k, P), :])
                for r in range(4):
                    nc.gpsimd.tensor_mul(
                        wkvb_t[k][:, ts(r, P)], wkvb_t[k][:, ts(r, P)],
                        skvbr_sb[:, r, k:k + 1].to_broadcast((P, P)))

            # o_proj weights last (needed at the end)
            wo_t = [pp.tile([P, HID], BF, tag=f"wo{k}", name=f"wo{k}")
                    for k in range(2)]
            for k in range(2):
                nc.scalar.dma_start(out=wo_t[k][:], in_=woT[ts(k, P), :])
                for j in range(16):
                    nc.gpsimd.tensor_mul(
                        wo_t[k][:, ts(j, P)], wo_t[k][:, ts(j, P)],
                        sor_sb[:, j, k:k + 1].to_broadcast((P, P)))

            qn = [[pp.tile([P, 1024], BF, tag=f"qn{h}_{hf}",
                           name=f"qn{h}_{hf}") for hf in range(2)]
                  for h in range(HPC)]
            qpe_all = pp.tile([P, S], BF, tag="qpe")  # rows 0:64 h0, 64:128 h1
            kn = [[pp.tile([P, SQB], BF, tag=f"kn{h}_{sq}",
                           name=f"kn{h}_{sq}") for sq in range(NSQB)]
                  for h in range(HPC)]
            v_t = [pp.tile([P, HPC * VDIM], BF, tag=f"v{t}", name=f"v{t}")
                   for t in range(NSKT)]
            attnT = [pp.tile([P, S], BF, tag=f"at{h}", name=f"at{h}")
                     for h in range(HPC)]

            # ---------- q_b projection (streamed over lnqT) ----------
            if "qb" in phases:
              with tc.tile_pool(name="lnqsp", bufs=8) as lnqsp, \
                 tc.tile_pool(name="psqb", bufs=1, space="PSUM") as psqb:
                for hf in range(2):
                    ps_mo = [psqb.tile([P, 1024], F32, tag=f"qb{mo}",
                                       name=f"psqb{mo}") for mo in range(3)]
                    for k in range(KQ):
                        lt = lnqsp.tile([P, 1024], BF, tag="lnqs")
                        nc.sync.dma_start(out=lt[:],
                                           in_=lnqT[ts(k, P), ts(hf, 1024)])
                        for mo in range(3):
                            for sq in range(2):
                                nc.tensor.matmul(
                                    ps_mo[mo][:, ts(sq, SQB)],
                                    wqb_t[k][:, ts(mo, P)],
                                    lt[:, ts(sq, SQB)],
                                    start=(k == 0), stop=(k == KQ - 1))
                    for h in range(HPC):
                        nc.vector.tensor_copy(qn[h][hf][:], ps_mo[h][:])
                    nc.scalar.copy(qpe_all[:, ts(hf, 1024)], ps_mo[2][:])

            # ---------- rope on q_pe ----------
            if "rope" in phases:
              with tc.tile_pool(name="ropep", bufs=1) as rp:
                qsw = rp.tile([P, S], BF, tag="qsw")
                for h in range(HPC):
                    o = h * ROPE
                    nc.sync.dma_start(out=qsw[o:o + H2, :],
                                      in_=qpe_all[o + H2:o + ROPE, :])
                    nc.sync.dma_start(out=qsw[o + H2:o + ROPE, :],
                                      in_=qpe_all[o:o + H2, :])
                rt = rp.tile([P, S], BF, tag="ropet")
                nc.vector.tensor_mul(rt[:], qpe_all[:], cos2_sb[:])
                ru = rp.tile([P, S], BF, tag="ropeu")
                nc.vector.tensor_mul(ru[:], qsw[:], sing2_sb[:])
                nc.vector.tensor_add(qpe_all[:], rt[:], ru[:])

            # ---------- kv_b projection ----------
            if "kv" in phases:
              with tc.tile_pool(name="pskv", bufs=2, space="PSUM") as pskvp:
                # interleaved by sq chunk so attention block b can start as
                # soon as its kn/v tiles land (per-block tiles keep deps tight)
                for sq in range(NSQB):
                    for h in range(HPC):
                        ps = pskvp.tile([P, SQB], F32, tag="pskn")
                        for k in range(4):
                            nc.tensor.matmul(ps[:], wkvb_t[k][:, ts(h, P)],
                                             lnkv_t[k][:, ts(sq, SQB)],
                                             start=(k == 0), stop=(k == 3))
                        nc.vector.tensor_copy(kn[h][sq][:], ps[:])
                    for t in range(4 * sq, 4 * sq + 4):
                        ps = pskvp.tile([P, HPC * VDIM], F32, tag="psv")
                        for k in range(4):
                            nc.tensor.matmul(ps[:], lnkv_t[k][:, ts(t, P)],
                                             wkvb_t[k][:, 2 * P:4 * P],
                                             start=(k == 0), stop=(k == 3))
                        nc.scalar.copy(v_t[t][:], ps[:])

            # ---------- attention ----------
            if "attn" in phases:
              with tc.tile_pool(name="probsp", bufs=8) as probsp, \
                 tc.tile_pool(name="sumsp", bufs=4) as sumsp, \
                 tc.tile_pool(name="recp", bufs=4) as recp, \
                 tc.tile_pool(name="pscp", bufs=3, space="PSUM") as pscp, \
                 tc.tile_pool(name="patp", bufs=2, space="PSUM") as patp:
                for h in range(HPC):
                    o = h * ROPE
                    for b in range(NSQB):
                        nsk = 4 * (b + 1)
                        ps_at = patp.tile([P, SQB], F32, tag="psat")
                        sumacc = sumsp.tile([P, SQB], F32R, tag="sumacc")
                        for t in range(nsk):
                            ps_s = pscp.tile([P, SQB], F32, tag="pss", bufs=5)
                            nc.tensor.matmul(
                                ps_s[:], kn[h][t // 4][:, ts(t % 4, P)],
                                qn[h][b // 2][:, ts(b % 2, SQB)],
                                start=True, stop=False)
                            nc.tensor.matmul(ps_s[:],
                                             kpe2_sb[o:o + ROPE, ts(t, P)],
                                             qpe_all[o:o + ROPE, ts(b, SQB)],
                                             start=False, stop=True)
                            pt = probsp.tile([P, SQB], BF, tag="probs")
                            nc.scalar.activation(pt[:], ps_s[:], AF.Exp,
                                                 bias=0.0, scale=SOFTMAX_SCALE)
                            d = t - 4 * b
                            if d >= 0:
                                nc.gpsimd.tensor_mul(
                                    pt[:], pt[:],
                                    maskw[:, 384 - 128 * d:896 - 128 * d])
                            if t == 0:
                                nc.vector.tensor_copy(sumacc[:], pt[:])
                            elif t % 2 == 1:
                                nc.vector.tensor_add(sumacc[:], sumacc[:], pt[:])
                            else:
                                nc.gpsimd.tensor_add(sumacc[:], sumacc[:], pt[:])
                            nc.tensor.matmul(ps_at[:], v_t[t][:, ts(h, VDIM)],
                                             pt[:],
                                             start=(t == 0), stop=(t == nsk - 1))
                        ps_sum = pscp.tile([P, SQB], F32, tag="pssum", bufs=1)
                        nc.tensor.matmul(ps_sum[:], ones_sb[:], sumacc[:],
                                         start=True, stop=True)
                        rec = recp.tile([P, SQB], F32, tag="rec")
                        nc.vector.reciprocal(rec[:], ps_sum[:])
                        nc.vector.tensor_mul(attnT[h][:, ts(b, SQB)],
                                             ps_at[:], rec[:])

            # ---------- o_proj partial: outT[o, s] = sum_pc wo[o,pc] attnT[pc,s]
            if "oproj" in phases:
              with tc.tile_pool(name="ostp", bufs=3) as ostp, \
                 tc.tile_pool(name="psop", bufs=2, space="PSUM") as psop:
                copy_eng = [nc.vector.tensor_copy, nc.scalar.copy]
                for mo in range(HID // P):
                    po = psop.tile([P, S], F32, tag="pso")
                    for k in range(HPC):
                        for sq in range(NSQB):
                            nc.tensor.matmul(po[:, ts(sq, SQB)],
                                             wo_t[k][:, ts(mo, P)],
                                             attnT[k][:, ts(sq, SQB)],
                                             start=(k == 0), stop=(k == HPC - 1))
                    ost = ostp.tile([P, S], BF, tag="ost")
                    for sq in range(NSQB):
                        copy_eng[(mo * NSQB + sq) % 2](ost[:, ts(sq, SQB)],
                                                       po[:, ts(sq, SQB)])
                    for dq in range(4):
                        nc.sync.dma_start(out=outT[ts(mo, P), ds(dq * 512, 512)],
                                          in_=ost[:, ts(dq, SQB)])
    nc.compile()
    return nc


# --------------------------------------------------------------------------
# Host orchestration
# --------------------------------------------------------------------------

_CACHE = {}
_LAST_L1_MAPS = None
_LAST_L2_MAPS = None


def _get(name, builder):
    if name not in _CACHE:
        _CACHE[name] = builder()
    return _CACHE[name]


class _SimResults:
    def __init__(self, results):
        self.results = results
        self.exec_time_ns = None


def _run(nc, in_maps, core_ids):
    if os.environ.get("BASS_KERNEL_SIM"):
        from concourse.bass_interp import CoreSim
        results = []
        out_names = [
            alloc.memorylocations[0].name
            for alloc in nc.m.functions[0].allocations
            if getattr(alloc, "kind", None) == "ExternalOutput"
            and getattr(alloc, "memorylocations", None)
        ]
        for in_map in in_maps:
            sim = CoreSim(nc, trace=False)
            for k, v in in_map.items():
                sim.tensor(k)[:] = v
            sim.simulate(check_with_hw=False)
            results.append({n: np.array(sim.tensor(n)) for n in out_names})
        return _SimResults(results)
    return run_bass_kernel_spmd(nc, in_maps, core_ids=core_ids)


def _c(a):
    return np.ascontiguousarray(a, dtype=np.float32)


def _b(a):
    return np.ascontiguousarray(np.asarray(a, dtype=np.float32).astype(BF_NP))


def run_l1(hidden_states, wq_a, sq_a, wkv_a, skv_a, q_ln_w, kv_ln_w, cos, sin):
    nc = _get("l1", build_l1)
    wqaT = _b(wq_a.T)
    wkvaT = _b(wkv_a.T)
    in_maps = []
    for c in range(NC_N):
        rows = slice(c * R, (c + 1) * R)
        in_maps.append({
            "xT": _b(hidden_states[rows].T),
            "wqaT": wqaT,
            "wkvaT": wkvaT,
            "sqa": _c(sq_a),
            "skva": _c(skv_a),
            "qlnw": _c(q_ln_w[None, :]),
            "kvlnw": _c(kv_ln_w[None, :]),
            "cosr": _c(cos[rows]),
            "sinr": _c(sin[rows]),
        })
    global _LAST_L1_MAPS
    _LAST_L1_MAPS = in_maps
    res = _run(nc, in_maps, list(range(NC_N)))
    lnq = np.concatenate([np.asarray(r["lnq"]) for r in res.results], axis=0)
    lnkv = np.concatenate([np.asarray(r["lnkv"]) for r in res.results], axis=0)
    kpe = np.concatenate([np.asarray(r["kpe"]) for r in res.results], axis=0)
    return lnq, lnkv, kpe


def _l2_weight_shards(c, wq_b, sq_b, wkv_b, skv_b, wo, so):
    h0, h1 = HPC * c, HPC * c + 1
    # wq_b rows reordered [nope_h0 | nope_h1 | pe_h0 | pe_h1]
    rows = np.concatenate([
        np.arange(h0 * HEAD, h0 * HEAD + NOPE),
        np.arange(h1 * HEAD, h1 * HEAD + NOPE),
        np.arange(h0 * HEAD + NOPE, (h0 + 1) * HEAD),
        np.arange(h1 * HEAD + NOPE, (h1 + 1) * HEAD),
    ])
    wqbT = _b(wq_b[rows].T)                      # [1536, 384]
    # run-constant scale table: runs [0:128,128:192,192:256,256:320,320:384]
    # hit original row-blocks [3c, 3c+1, 3c+2, 3c+1, 3c+2]
    run_blk = [3 * c, 3 * c + 1, 3 * c + 2, 3 * c + 1, 3 * c + 2]
    sqbr = _c(sq_b[run_blk, :])                  # [5, 12]

    # wkv_b rows reordered [kn_h0 | kn_h1 | v_h0 | v_h1]
    krows = np.concatenate([
        np.arange(h0 * (NOPE + VDIM), h0 * (NOPE + VDIM) + NOPE),
        np.arange(h1 * (NOPE + VDIM), h1 * (NOPE + VDIM) + NOPE),
        np.arange(h0 * (NOPE + VDIM) + NOPE, (h0 + 1) * (NOPE + VDIM)),
        np.arange(h1 * (NOPE + VDIM) + NOPE, (h1 + 1) * (NOPE + VDIM)),
    ])
    wkvbT = _b(wkv_b[krows].T)                   # [512, 512]
    # runs of 128 hit original row-blocks [4c, 4c+2, 4c+1, 4c+3]
    kv_run_blk = [4 * c, 4 * c + 2, 4 * c + 1, 4 * c + 3]
    skvbr = _c(skv_b[kv_run_blk, :])             # [4, 4]

    cols = np.concatenate([np.arange(h0 * VDIM, (h0 + 1) * VDIM),
                           np.arange(h1 * VDIM, (h1 + 1) * VDIM)])
    woT = _b(wo[:, cols].T)                      # [256, 2048]
    # sor[j, kk] = so[out-block j, in-block of head kk]
    sor = _c(so[:, [2 * c, 2 * c + 1]])          # [16, 2]
    return wqbT, sqbr, wkvbT, skvbr, woT, sor


def run_l2(lnq, lnkv, kpe, cos, sin, wq_b, sq_b, wkv_b, skv_b, wo, so):
    nc = _get("l2", build_l2)
    lnqT = np.ascontiguousarray(np.asarray(lnq).T)
    lnkvT = np.ascontiguousarray(np.asarray(lnkv).T)
    kpeT = np.ascontiguousarray(np.asarray(kpe).T)
    cosT = _b(cos.T)
    sinT = _b(sin.T)
    ones = np.ones((P, P), dtype=np.float32)
    in_maps = []
    for c in range(NC_N):
        wqbT, sqbr, wkvbT, skvbr, woT, sor = _l2_weight_shards(
            c, wq_b, sq_b, wkv_b, skv_b, wo, so)
        in_maps.append({
            "lnqT": lnqT, "lnkvT": lnkvT, "kpeT": kpeT,
            "cosT": cosT, "sinT": sinT,
            "wqbT": wqbT, "sqbr": sqbr,
            "wkvbT": wkvbT, "skvbr": skvbr,
            "woT": woT, "sor": sor,
            "ones": ones,
        })
    global _LAST_L2_MAPS
    _LAST_L2_MAPS = in_maps
    res = _run(nc, in_maps, list(range(NC_N)))
    acc = np.asarray(res.results[0]["outT"]).astype(np.float32)
    for c in range(1, NC_N):
        acc = acc + np.asarray(res.results[c]["outT"]).astype(np.float32)
    return _c(acc.T)


def kernel(hidden_states, cos, sin, wq_a, sq_a, wq_b, sq_b, wkv_a, skv_a,
           wkv_b, skv_b, wo, so, q_ln_w, kv_ln_w):
    lnq, lnkv, kpe = run_l1(hidden_states, wq_a, sq_a, wkv_a, skv_a,
                            q_ln_w, kv_ln_w, cos, sin)
    return run_l2(lnq, lnkv, kpe, cos, sin, wq_b, sq_b, wkv_b, skv_b, wo, so)
